# revision 7
# baseline (speedup 1.0000x reference)
"""Binary depthwise 3x3 conv (SAME padding) on 8 Trainium2 NeuronCores.

Problem: x (16,112,112,384) f32, w (3,3,384,1) f32.
out[n,h,w,c] = sum_{dy,dx} sign(clip(w))[dy,dx,c] * x[n,h+dy-1,w+dx-1,c]

Strategy (data-parallel, 2 images per core):
  - DMA x in natural NHWC layout (contiguous per partition).
  - PE transpose-mode flips [spatial, c] -> [c, spatial] into a zero-padded
    114-stride row layout so all 9 taps become uniform AP offsets.
  - 7 taps run as float32r diag-matmuls on the PE accumulating in PSUM;
    2 taps run on DVE (tensor_scalar mult + scalar_tensor_tensor), merged
    with the PSUM partial by a final scalar_tensor_tensor.
  - PE transposes back to [spatial, c]; ACT evicts PSUM->SBUF; DMA out.
"""

import os
import sys

sys.path.insert(0, "/opt/trn_rl_repo")

import numpy as np

import concourse.bacc as bacc
import concourse.mybir as mybir
from concourse.tile import TileContext
from concourse.bass_utils import run_bass_kernel_spmd

F32 = mybir.dt.float32
F32R = mybir.dt.float32r

N_CORES = 8
B, H, W, C = 16, 112, 112, 384
IMG_PER_CORE = B // N_CORES          # 2
S = H * W                            # 12544 spatial positions per image
ROWS_PER_CORE = IMG_PER_CORE * S     # 25088
P = 128
CBLK = C // P                        # 3 channel blocks
WP = 114                             # padded row stride (w = -1 .. 112)
HP = 114                             # padded rows (h = -1 .. 112)
ROWG = 8                             # rows per transpose/evict group (8*112 = 7*128)
CHUNKS_PER_G = ROWG * W // P         # 7
NG = H // ROWG                       # 14 row groups per image
DMA_GROUPS = 7                       # input DMAs per (img, cblk): 16 rows each
ROWS_PER_DMA = H // DMA_GROUPS       # 16
CHUNKS_PER_DMA = ROWS_PER_DMA * W // P  # 14
TAP_ROWS = 4                         # output rows per tap matmul (N = 448)
NHG = H // TAP_ROWS                  # 28 tap groups per (img, cblk)

TAPS = [(dy, dx) for dy in (-1, 0, 1) for dx in (-1, 0, 1)]
DVE_TAPS = [(1, 0), (1, 1)]
PE_TAPS = [t for t in TAPS if t not in DVE_TAPS]


def _tap_idx(dy, dx):
    return (dy + 1) * 3 + (dx + 1)


def build_bass():
    nc = bacc.Bacc(
        "TRN2", target_bir_lowering=False, debug=False, num_devices=N_CORES
    )
    x_d = nc.dram_tensor("x", [ROWS_PER_CORE, C], F32, kind="ExternalInput").ap()
    # float32r end-to-end for the PE-tap operands: the BIR verifier requires
    # every producer of fp32r-matmul data to round to fp32r.
    diag_d = nc.dram_tensor(
        "diag", [P, 9 * CBLK * P], F32R, kind="ExternalInput"
    ).ap()
    signs_d = nc.dram_tensor("signs", [P, 9 * CBLK], F32, kind="ExternalInput").ap()
    ident_d = nc.dram_tensor("ident", [P, P], F32, kind="ExternalInput").ap()
    out_d = nc.dram_tensor("out", [ROWS_PER_CORE, C], F32, kind="ExternalOutput").ap()

    with TileContext(nc) as tc:
        with (
            tc.tile_pool(name="const", bufs=1) as const_pool,
            tc.tile_pool(name="xnat", bufs=3) as xnat_pool,
            tc.tile_pool(name="xT", bufs=2) as xT_pool,
            tc.tile_pool(name="acc", bufs=2) as acc_pool,
            tc.tile_pool(name="stag", bufs=2) as stag_pool,
            tc.tile_pool(name="onat", bufs=3) as onat_pool,
            tc.tile_pool(name="tinp", bufs=2, space="PSUM") as tin_psum,
            tc.tile_pool(name="tapp", bufs=2, space="PSUM") as tap_psum,
            tc.tile_pool(name="toutp", bufs=2, space="PSUM") as tout_psum,
        ):
            diag_sb = const_pool.tile([P, 9 * CBLK * P], F32R)
            nc.sync.dma_start(diag_sb[:], diag_d)
            signs_sb = const_pool.tile([P, 9 * CBLK], F32)
            nc.sync.dma_start(signs_sb[:], signs_d)
            ident_sb = const_pool.tile([P, P], F32)
            nc.sync.dma_start(ident_sb[:], ident_d)

            for img in range(IMG_PER_CORE):
                for b in range(CBLK):
                    _unit(
                        nc, tc, img, b,
                        x_d, out_d, diag_sb, signs_sb, ident_sb,
                        xnat_pool, xT_pool, acc_pool, stag_pool, onat_pool,
                        tin_psum, tap_psum, tout_psum,
                    )
    nc.finalize()
    return nc


def _unit(
    nc, tc, img, b,
    x_d, out_d, diag_sb, signs_sb, ident_sb,
    xnat_pool, xT_pool, acc_pool, stag_pool, onat_pool,
    tin_psum, tap_psum, tout_psum,
):
    row0 = img * S

    # ---- transposed, zero-padded x for this (image, channel block) ----
    # Stored as float32r (the PE-tap moving operand must be fp32r-rounded by
    # its producers); DVE taps read it through a plain-f32 bitcast view.
    xT = xT_pool.tile([P, HP * WP], F32R, tag="xT")
    xT3 = xT.rearrange("p (r w) -> p r w", w=WP)  # [128, 114, 114]
    xT3f = xT.bitcast(F32).rearrange("p (r w) -> p r w", w=WP)
    # zero the pad slots: top pad row (h=-1), bottom pad row (h=112),
    # and the two w-pad slots of every row (contiguous pairs at 114k+113).
    # memset can't encode an f32r value in ISA -> write the zero bits as u32.
    xTu3 = xT.bitcast(mybir.dt.uint32).rearrange("p (r w) -> p r w", w=WP)
    nc.vector.memset(xTu3[:, 0, :], 0)
    nc.vector.memset(xTu3[:, HP - 1, :], 0)
    wpads = xT.bitcast(mybir.dt.uint32)[
        :, WP - 1 : WP - 1 + (HP - 1) * WP
    ].rearrange("p (r t) -> p r t", t=WP)[:, :, 0:2]
    nc.vector.memset(wpads, 0)

    # ---- load + transpose-in ----
    for dg in range(DMA_GROUPS):
        xnat = xnat_pool.tile([P, CHUNKS_PER_DMA, P], F32, tag="xnat")
        src = x_d[
            row0 + dg * ROWS_PER_DMA * W : row0 + (dg + 1) * ROWS_PER_DMA * W,
            b * P : (b + 1) * P,
        ].rearrange("(k p) c -> p k c", p=P)
        nc.sync.dma_start(xnat[:], src)
        for half in range(2):
            g = dg * 2 + half  # row group (8 rows) index, 0..13
            ps_tin = tin_psum.tile([P, ROWG * W], F32, tag="tin")
            for k in range(CHUNKS_PER_G):
                ck = half * CHUNKS_PER_G + k
                nc.tensor.transpose(
                    ps_tin[:, k * P : (k + 1) * P], xnat[:, ck, :], ident_sb[:]
                )
            # evict into padded rows 8g..8g+7 (padded row index 8g+1..8g+9)
            dst = xT3[:, g * ROWG + 1 : g * ROWG + 1 + ROWG, 1 : 1 + W]
            src_v = ps_tin.rearrange("p (r w) -> p r w", w=W)
            nc.scalar.copy(dst, src_v)

    # ---- taps ----
    stag = None
    for hg in range(NHG):
        h0 = hg * TAP_ROWS
        ps_acc = tap_psum.tile([P, TAP_ROWS * W], F32, tag="tap")
        for i, (dy, dx) in enumerate(PE_TAPS):
            t = _tap_idx(dy, dx)
            lhsT = diag_sb[:, (t * CBLK + b) * P : (t * CBLK + b + 1) * P]
            rhs = xT3[
                :, h0 + 1 + dy : h0 + 1 + dy + TAP_ROWS, 1 + dx : 1 + dx + W
            ]
            nc.tensor.matmul(
                ps_acc[:],
                lhsT,
                rhs,
                start=(i == 0),
                stop=(i == len(PE_TAPS) - 1),
            )
        acc = acc_pool.tile([P, TAP_ROWS * W], F32, tag="acc")
        accv = acc.rearrange("p (r w) -> p r w", w=W)
        for i, (dy, dx) in enumerate(DVE_TAPS):
            t = _tap_idx(dy, dx)
            sg = signs_sb[:, t * CBLK + b : t * CBLK + b + 1]
            xs = xT3f[
                :, h0 + 1 + dy : h0 + 1 + dy + TAP_ROWS, 1 + dx : 1 + dx + W
            ]
            if i == 0:
                nc.vector.tensor_scalar(
                    accv, xs, sg, None, mybir.AluOpType.mult
                )
            else:
                nc.vector.scalar_tensor_tensor(
                    accv, xs, sg, accv,
                    mybir.AluOpType.mult, mybir.AluOpType.add,
                )
        # merge PE partial (PSUM) + DVE partial into the staging tile
        if hg % 2 == 0:
            stag = stag_pool.tile([P, 2 * TAP_ROWS * W], F32, tag="stag")
        half = hg % 2
        nc.vector.scalar_tensor_tensor(
            stag[:, half * TAP_ROWS * W : (half + 1) * TAP_ROWS * W],
            ps_acc[:],
            1.0,
            acc[:],
            mybir.AluOpType.mult,
            mybir.AluOpType.add,
        )
        # ---- transpose-out + evict + store per 8 output rows ----
        if half == 1:
            g = hg // 2
            onat = onat_pool.tile([P, CHUNKS_PER_G, P], F32, tag="onat")
            for batch, nchunk in ((0, 4), (1, 3)):
                ps_out = tout_psum.tile([P, 4 * P], F32, tag="tout")
                for k in range(nchunk):
                    ck = batch * 4 + k
                    nc.tensor.transpose(
                        ps_out[:, k * P : (k + 1) * P],
                        stag[:, ck * P : (ck + 1) * P],
                        ident_sb[:],
                    )
                nc.scalar.copy(
                    onat[:, batch * 4 : batch * 4 + nchunk, :],
                    ps_out[:, : nchunk * P].rearrange("p (k c) -> p k c", c=P),
                )
            dst = out_d[
                row0 + g * ROWG * W : row0 + (g + 1) * ROWG * W,
                b * P : (b + 1) * P,
            ].rearrange("(k p) c -> p k c", p=P)
            nc.sync.dma_start(dst, onat[:])


_NC_CACHE = None


def _get_nc():
    global _NC_CACHE
    if _NC_CACHE is None:
        _NC_CACHE = build_bass()
    return _NC_CACHE


def _host_inputs(w):
    """Per-core constant tensors derived from w (shared by all cores)."""
    signs = np.sign(np.clip(w.astype(np.float32), -1.0, 1.0))[:, :, :, 0]  # (3,3,384)
    signs_flat = signs.reshape(9, C)  # tap-major
    diag = np.zeros((P, 9 * CBLK * P), dtype=np.float32)
    signs_in = np.zeros((P, 9 * CBLK), dtype=np.float32)
    for t in range(9):
        for b in range(CBLK):
            sv = signs_flat[t, b * P : (b + 1) * P]
            col0 = (t * CBLK + b) * P
            diag[np.arange(P), col0 + np.arange(P)] = sv
            signs_in[:, t * CBLK + b] = sv
    ident = np.eye(P, dtype=np.float32)
    return diag, signs_in, ident


def kernel(x, w):
    x = np.asarray(x, dtype=np.float32)
    w = np.asarray(w, dtype=np.float32)
    assert x.shape == (B, H, W, C), x.shape
    nc = _get_nc()
    diag, signs_in, ident = _host_inputs(w)
    in_maps = []
    for core in range(N_CORES):
        xc = x[core * IMG_PER_CORE : (core + 1) * IMG_PER_CORE]
        in_maps.append(
            {
                "x": np.ascontiguousarray(xc.reshape(ROWS_PER_CORE, C)),
                "diag": diag,
                "signs": signs_in,
                "ident": ident,
            }
        )
    res = run_bass_kernel_spmd(nc, in_maps, core_ids=list(range(N_CORES)))
    out = np.empty((B, H, W, C), dtype=np.float32)
    for core in range(N_CORES):
        out[core * IMG_PER_CORE : (core + 1) * IMG_PER_CORE] = res.results[core][
            "out"
        ].reshape(IMG_PER_CORE, H, W, C)
    return out


if __name__ == "__main__":
    rng = np.random.default_rng(0)
    x = rng.standard_normal((B, H, W, C), dtype=np.float32)
    w = rng.standard_normal((3, 3, C, 1), dtype=np.float32)
    out = kernel(x, w)
    print("out", out.shape, out.dtype, float(np.abs(out).mean()))


# revision 29
# speedup vs baseline: 9.4931x; 9.4931x over previous
"""Binary depthwise 3x3 conv (SAME padding) on 8 Trainium2 NeuronCores.

Problem: x (16,112,112,384) f32, w (3,3,384,1) f32.
out[n,h,w,c] = sum_{dy,dx} sign(clip(w))[dy,dx,c] * x[n,h+dy-1,w+dx-1,c]

Strategy (data-parallel, 2 images per core):
  - DMA x in natural NHWC layout (contiguous per partition).
  - PE transpose-mode flips [spatial, c] -> [c, spatial] into a zero-padded
    114-stride row layout so all 9 taps become uniform AP offsets.
  - 7 taps run as float32r diag-matmuls on the PE accumulating in PSUM;
    2 taps run on DVE (tensor_scalar mult + scalar_tensor_tensor), merged
    with the PSUM partial by a final scalar_tensor_tensor.
  - PE transposes back to [spatial, c]; ACT evicts PSUM->SBUF; DMA out.
"""

import os
import sys

sys.path.insert(0, "/opt/trn_rl_repo")

import numpy as np

import concourse.bacc as bacc
import concourse.mybir as mybir
from concourse.tile import TileContext
from concourse.bass_utils import run_bass_kernel_spmd

F32 = mybir.dt.float32
F32R = mybir.dt.float32r

N_CORES = 8
B, H, W, C = 16, 112, 112, 384
IMG_PER_CORE = B // N_CORES          # 2
S = H * W                            # 12544 spatial positions per image
ROWS_PER_CORE = IMG_PER_CORE * S     # 25088
P = 128
CBLK = C // P                        # 3 channel blocks
WP = 114                             # padded row stride (w = -1 .. 112)
HP = 114                             # padded rows (h = -1 .. 112)
ROWG = 8                             # rows per transpose/evict group (8*112 = 7*128)
CHUNKS_PER_G = ROWG * W // P         # 7
NG = H // ROWG                       # 14 row groups per image
DMA_GROUPS = 7                       # input DMAs per (img, cblk): 16 rows each
ROWS_PER_DMA = H // DMA_GROUPS       # 16
CHUNKS_PER_DMA = ROWS_PER_DMA * W // P  # 14
TAP_ROWS = 4                         # output rows per tap matmul (N = 448)
NHG = H // TAP_ROWS                  # 28 tap groups per (img, cblk)

TAPS = [(dy, dx) for dy in (-1, 0, 1) for dx in (-1, 0, 1)]

# tunables (overridable via build_bass kwargs); defaults = best measured
# config from the TimelineSim sweep (333.7 us predicted vs 376 initial)
DEFAULT_CFG = dict(
    n_dve_taps=3,      # taps on DVE (rest on PE); int or per-hg list (cycled)
    act_first_mult=True,   # first DVE tap multiply on ACT instead of DVE
    out_dma_on_act=False,  # issue output DMAs on the ACT HWDGE ring
    tout_single=False,     # single 7-chunk T_out psum buffer + one big evict
    f32r_transpose=True,   # run PE transposes in float32r (1.5 vs 2 cyc/row)
    dve_inplace=False,     # DVE taps RMW directly into the tap PSUM bank;
                           # ACT evicts PSUM->staging (no DVE merge op)
    dve8=True,             # run DVE taps at 8-row granularity (amortize
                           # per-op overhead across two tap groups)
)


def _tap_idx(dy, dx):
    return (dy + 1) * 3 + (dx + 1)


def build_bass(reps=1, **cfg_over):
    cfg = {**DEFAULT_CFG, **cfg_over}
    tdt = F32R if cfg["f32r_transpose"] else F32
    nc = bacc.Bacc(
        "TRN2", target_bir_lowering=False, debug=False, num_devices=N_CORES
    )
    x_d = nc.dram_tensor("x", [ROWS_PER_CORE, C], tdt, kind="ExternalInput").ap()
    # float32r end-to-end for the PE-tap operands: the BIR verifier requires
    # every producer of fp32r-matmul data to round to fp32r.
    diag_d = nc.dram_tensor(
        "diag", [P, 9 * CBLK * P], F32R, kind="ExternalInput"
    ).ap()
    signs_d = nc.dram_tensor("signs", [P, 9 * CBLK], F32, kind="ExternalInput").ap()
    ident_d = nc.dram_tensor("ident", [P, P], tdt, kind="ExternalInput").ap()
    out_d = nc.dram_tensor("out", [ROWS_PER_CORE, C], tdt, kind="ExternalOutput").ap()

    with TileContext(nc) as tc:
        with (
            tc.tile_pool(name="const", bufs=1) as const_pool,
            tc.tile_pool(name="xnat", bufs=3) as xnat_pool,
            tc.tile_pool(name="xT", bufs=2) as xT_pool,
            tc.tile_pool(name="acc", bufs=2) as acc_pool,
            tc.tile_pool(name="stag", bufs=2) as stag_pool,
            tc.tile_pool(name="onat", bufs=3) as onat_pool,
            tc.tile_pool(name="tinp", bufs=2, space="PSUM") as tin_psum,
            tc.tile_pool(name="tapp", bufs=2, space="PSUM") as tap_psum,
            tc.tile_pool(
                name="toutp", bufs=1 if cfg["tout_single"] else 2, space="PSUM"
            ) as tout_psum,
        ):
            diag_sb = const_pool.tile([P, 9 * CBLK * P], F32R)
            nc.sync.dma_start(diag_sb[:], diag_d)
            signs_sb = const_pool.tile([P, 9 * CBLK], F32)
            nc.sync.dma_start(signs_sb[:], signs_d)
            ident_sb = const_pool.tile([P, P], tdt)
            nc.sync.dma_start(ident_sb[:], ident_d)

            for _rep in range(reps):
                for img in range(IMG_PER_CORE):
                    for b in range(CBLK):
                        _unit(
                            nc, tc, img, b,
                            x_d, out_d, diag_sb, signs_sb, ident_sb,
                            xnat_pool, xT_pool, acc_pool, stag_pool, onat_pool,
                            tin_psum, tap_psum, tout_psum, cfg,
                        )
    nc.finalize()
    return nc


def _unit(
    nc, tc, img, b,
    x_d, out_d, diag_sb, signs_sb, ident_sb,
    xnat_pool, xT_pool, acc_pool, stag_pool, onat_pool,
    tin_psum, tap_psum, tout_psum, cfg,
):
    nd = cfg["n_dve_taps"]
    nd_list = [nd] * NHG if isinstance(nd, int) else [
        nd[i % len(nd)] for i in range(NHG)
    ]
    tdt = F32R if cfg["f32r_transpose"] else F32
    row0 = img * S

    # ---- transposed, zero-padded x for this (image, channel block) ----
    # Stored as float32r (the PE-tap moving operand must be fp32r-rounded by
    # its producers); DVE taps read it through a plain-f32 bitcast view.
    xT = xT_pool.tile([P, HP * WP], F32R, tag="xT")
    xT3 = xT.rearrange("p (r w) -> p r w", w=WP)  # [128, 114, 114]
    xT3f = xT.bitcast(F32).rearrange("p (r w) -> p r w", w=WP)
    # zero the pad slots: top pad row (h=-1), bottom pad row (h=112),
    # and the two w-pad slots of every row (contiguous pairs at 114k+113).
    # memset can't encode an f32r value in ISA -> write the zero bits as u32.
    xTu3 = xT.bitcast(mybir.dt.uint32).rearrange("p (r w) -> p r w", w=WP)
    nc.vector.memset(xTu3[:, 0, :], 0)
    nc.vector.memset(xTu3[:, HP - 1, :], 0)
    wpads = xT.bitcast(mybir.dt.uint32)[
        :, WP - 1 : WP - 1 + (HP - 1) * WP
    ].rearrange("p (r t) -> p r t", t=WP)[:, :, 0:2]
    nc.vector.memset(wpads, 0)

    # ---- load + transpose-in ----
    for dg in range(DMA_GROUPS):
        xnat = xnat_pool.tile([P, CHUNKS_PER_DMA, P], tdt, tag="xnat")
        src = x_d[
            row0 + dg * ROWS_PER_DMA * W : row0 + (dg + 1) * ROWS_PER_DMA * W,
            b * P : (b + 1) * P,
        ].rearrange("(k p) c -> p k c", p=P)
        nc.sync.dma_start(xnat[:], src)
        for half in range(2):
            g = dg * 2 + half  # row group (8 rows) index, 0..13
            ps_tin = tin_psum.tile([P, ROWG * W], tdt, tag="tin")
            for k in range(CHUNKS_PER_G):
                ck = half * CHUNKS_PER_G + k
                nc.tensor.transpose(
                    ps_tin[:, k * P : (k + 1) * P], xnat[:, ck, :], ident_sb[:]
                )
            # evict into padded rows 8g..8g+7 (padded row index 8g+1..8g+9)
            dst = xT3[:, g * ROWG + 1 : g * ROWG + 1 + ROWG, 1 : 1 + W]
            src_v = ps_tin.rearrange("p (r w) -> p r w", w=W)
            nc.scalar.copy(dst, src_v)

    # ---- taps ----
    stag = None
    for hg in range(NHG):
        n_dve = nd_list[hg]
        dve_taps = TAPS[9 - n_dve :]
        pe_taps = TAPS[: 9 - n_dve]
        h0 = hg * TAP_ROWS
        ps_acc = tap_psum.tile([P, TAP_ROWS * W], F32, tag="tap")
        for i, (dy, dx) in enumerate(pe_taps):
            t = _tap_idx(dy, dx)
            lhsT = diag_sb[:, (t * CBLK + b) * P : (t * CBLK + b + 1) * P]
            rhs = xT3[
                :, h0 + 1 + dy : h0 + 1 + dy + TAP_ROWS, 1 + dx : 1 + dx + W
            ]
            nc.tensor.matmul(
                ps_acc[:],
                lhsT,
                rhs,
                start=(i == 0),
                stop=(i == len(pe_taps) - 1),
            )
        if hg % 2 == 0:
            stag = stag_pool.tile([P, 2 * TAP_ROWS * W], tdt, tag="stag")
        half = hg % 2
        stag_slice = stag[:, half * TAP_ROWS * W : (half + 1) * TAP_ROWS * W]

        def dve_tap_views(i, rows=TAP_ROWS, hh=None):
            dy, dx = dve_taps[i]
            t = _tap_idx(dy, dx)
            sg = signs_sb[:, t * CBLK + b : t * CBLK + b + 1]
            hs = h0 if hh is None else hh
            xs = xT3f[
                :, hs + 1 + dy : hs + 1 + dy + rows, 1 + dx : 1 + dx + W
            ]
            return sg, xs

        if cfg["dve8"] and n_dve > 0:
            # 8-row DVE partial computed once per hg pair
            if half == 0:
                acc8 = acc_pool.tile([P, 2 * TAP_ROWS * W], F32, tag="acc")
                acc8v = acc8.rearrange("p (r w) -> p r w", w=W)
                _unit._acc8 = acc8  # stash on fn (single-threaded build)
                for i in range(n_dve):
                    sg, xs = dve_tap_views(i, rows=2 * TAP_ROWS)
                    if i == 0:
                        if cfg["act_first_mult"]:
                            nc.scalar.mul(acc8v, xs, sg)
                        else:
                            nc.vector.tensor_scalar(
                                acc8v, xs, sg, None, mybir.AluOpType.mult
                            )
                    else:
                        nc.vector.scalar_tensor_tensor(
                            acc8v, xs, sg, acc8v,
                            mybir.AluOpType.mult, mybir.AluOpType.add,
                        )
            acc8 = _unit._acc8
            nc.vector.scalar_tensor_tensor(
                stag_slice,
                ps_acc[:],
                1.0,
                acc8[:, half * TAP_ROWS * W : (half + 1) * TAP_ROWS * W],
                mybir.AluOpType.mult,
                mybir.AluOpType.add,
            )
        elif n_dve == 0:
            # no DVE partial: evict PSUM straight into staging on ACT
            nc.scalar.copy(stag_slice, ps_acc[:])
        elif cfg["dve_inplace"]:
            # DVE taps read-modify-write the PSUM accumulator after the PE
            # group completes; ACT does the final eviction into staging.
            for i in range(n_dve):
                sg, xs = dve_tap_views(i)
                nc.vector.scalar_tensor_tensor(
                    ps_acc[:], xs, sg, ps_acc[:],
                    mybir.AluOpType.mult, mybir.AluOpType.add,
                )
            nc.scalar.copy(stag_slice, ps_acc[:])
        else:
            acc = acc_pool.tile([P, TAP_ROWS * W], F32, tag="acc")
            accv = acc.rearrange("p (r w) -> p r w", w=W)
            for i in range(n_dve):
                sg, xs = dve_tap_views(i)
                if i == 0:
                    if cfg["act_first_mult"]:
                        nc.scalar.mul(accv, xs, sg)
                    else:
                        nc.vector.tensor_scalar(
                            accv, xs, sg, None, mybir.AluOpType.mult
                        )
                else:
                    nc.vector.scalar_tensor_tensor(
                        accv, xs, sg, accv,
                        mybir.AluOpType.mult, mybir.AluOpType.add,
                    )
            # merge PE partial (PSUM) + DVE partial into the staging tile
            nc.vector.scalar_tensor_tensor(
                stag_slice,
                ps_acc[:],
                1.0,
                acc[:],
                mybir.AluOpType.mult,
                mybir.AluOpType.add,
            )
        # ---- transpose-out + evict + store per 8 output rows ----
        if half == 1:
            g = hg // 2
            onat = onat_pool.tile([P, CHUNKS_PER_G, P], tdt, tag="onat")
            if cfg["tout_single"]:
                ps_out = tout_psum.tile([P, CHUNKS_PER_G * P], tdt, tag="tout")
                for k in range(CHUNKS_PER_G):
                    nc.tensor.transpose(
                        ps_out[:, k * P : (k + 1) * P],
                        stag[:, k * P : (k + 1) * P],
                        ident_sb[:],
                    )
                nc.scalar.copy(
                    onat[:],
                    ps_out[:].rearrange("p (k c) -> p k c", c=P),
                )
            else:
                for batch, nchunk in ((0, 4), (1, 3)):
                    ps_out = tout_psum.tile([P, 4 * P], tdt, tag="tout")
                    for k in range(nchunk):
                        ck = batch * 4 + k
                        nc.tensor.transpose(
                            ps_out[:, k * P : (k + 1) * P],
                            stag[:, ck * P : (ck + 1) * P],
                            ident_sb[:],
                        )
                    nc.scalar.copy(
                        onat[:, batch * 4 : batch * 4 + nchunk, :],
                        ps_out[:, : nchunk * P].rearrange("p (k c) -> p k c", c=P),
                    )
            dst = out_d[
                row0 + g * ROWG * W : row0 + (g + 1) * ROWG * W,
                b * P : (b + 1) * P,
            ].rearrange("(k p) c -> p k c", p=P)
            if cfg["out_dma_on_act"]:
                nc.scalar.dma_start(dst, onat[:])
            else:
                nc.sync.dma_start(dst, onat[:])


_NC_CACHE = None


def _get_nc():
    global _NC_CACHE
    if _NC_CACHE is None:
        _NC_CACHE = build_bass()
    return _NC_CACHE


def _host_inputs(w):
    """Per-core constant tensors derived from w (shared by all cores)."""
    signs = np.sign(np.clip(w.astype(np.float32), -1.0, 1.0))[:, :, :, 0]  # (3,3,384)
    signs_flat = signs.reshape(9, C)  # tap-major
    diag = np.zeros((P, 9 * CBLK * P), dtype=np.float32)
    signs_in = np.zeros((P, 9 * CBLK), dtype=np.float32)
    for t in range(9):
        for b in range(CBLK):
            sv = signs_flat[t, b * P : (b + 1) * P]
            col0 = (t * CBLK + b) * P
            diag[np.arange(P), col0 + np.arange(P)] = sv
            signs_in[:, t * CBLK + b] = sv
    ident = np.eye(P, dtype=np.float32)
    return diag, signs_in, ident


def kernel(x, w):
    x = np.asarray(x, dtype=np.float32)
    w = np.asarray(w, dtype=np.float32)
    assert x.shape == (B, H, W, C), x.shape
    nc = _get_nc()
    diag, signs_in, ident = _host_inputs(w)
    in_maps = []
    for core in range(N_CORES):
        xc = x[core * IMG_PER_CORE : (core + 1) * IMG_PER_CORE]
        in_maps.append(
            {
                "x": np.ascontiguousarray(xc.reshape(ROWS_PER_CORE, C)),
                "diag": diag,
                "signs": signs_in,
                "ident": ident,
            }
        )
    res = run_bass_kernel_spmd(nc, in_maps, core_ids=list(range(N_CORES)))
    out = np.empty((B, H, W, C), dtype=np.float32)
    for core in range(N_CORES):
        out[core * IMG_PER_CORE : (core + 1) * IMG_PER_CORE] = res.results[core][
            "out"
        ].reshape(IMG_PER_CORE, H, W, C)
    return out


if __name__ == "__main__":
    rng = np.random.default_rng(0)
    x = rng.standard_normal((B, H, W, C), dtype=np.float32)
    w = rng.standard_normal((3, 3, C, 1), dtype=np.float32)
    out = kernel(x, w)
    print("out", out.shape, out.dtype, float(np.abs(out).mean()))


# revision 32
# speedup vs baseline: 10.1282x; 1.0669x over previous
"""Binary depthwise 3x3 conv (SAME padding) on 8 Trainium2 NeuronCores.

Problem: x (16,112,112,384) f32, w (3,3,384,1) f32.
out[n,h,w,c] = sum_{dy,dx} sign(clip(w))[dy,dx,c] * x[n,h+dy-1,w+dx-1,c]

Strategy (data-parallel, 2 images per core):
  - DMA x in natural NHWC layout (contiguous per partition).
  - PE transpose-mode flips [spatial, c] -> [c, spatial] into a zero-padded
    114-stride row layout so all 9 taps become uniform AP offsets.
  - 7 taps run as float32r diag-matmuls on the PE accumulating in PSUM;
    2 taps run on DVE (tensor_scalar mult + scalar_tensor_tensor), merged
    with the PSUM partial by a final scalar_tensor_tensor.
  - PE transposes back to [spatial, c]; ACT evicts PSUM->SBUF; DMA out.
"""

import os
import sys

sys.path.insert(0, "/opt/trn_rl_repo")

import numpy as np

import concourse.bacc as bacc
import concourse.mybir as mybir
from concourse.tile import TileContext
from concourse.bass_utils import run_bass_kernel_spmd

F32 = mybir.dt.float32
F32R = mybir.dt.float32r

N_CORES = 8
B, H, W, C = 16, 112, 112, 384
IMG_PER_CORE = B // N_CORES          # 2
S = H * W                            # 12544 spatial positions per image
ROWS_PER_CORE = IMG_PER_CORE * S     # 25088
P = 128
CBLK = C // P                        # 3 channel blocks
WP = 114                             # padded row stride (w = -1 .. 112)
HP = 114                             # padded rows (h = -1 .. 112)
ROWG = 8                             # rows per transpose/evict group (8*112 = 7*128)
CHUNKS_PER_G = ROWG * W // P         # 7
NG = H // ROWG                       # 14 row groups per image
DMA_GROUPS = 7                       # input DMAs per (img, cblk): 16 rows each
ROWS_PER_DMA = H // DMA_GROUPS       # 16
CHUNKS_PER_DMA = ROWS_PER_DMA * W // P  # 14
TAP_ROWS = 4                         # output rows per tap matmul (N = 448)
NHG = H // TAP_ROWS                  # 28 tap groups per (img, cblk)

TAPS = [(dy, dx) for dy in (-1, 0, 1) for dx in (-1, 0, 1)]

# tunables (overridable via build_bass kwargs); defaults = best measured
# config from the TimelineSim sweep (333.7 us predicted vs 376 initial)
DEFAULT_CFG = dict(
    n_dve_taps=3,      # taps on DVE (rest on PE); int or per-hg list (cycled)
    act_first_mult=True,   # first DVE tap multiply on ACT instead of DVE
    out_dma_on_act=False,  # issue output DMAs on the ACT HWDGE ring
    tout_single=False,     # single 7-chunk T_out psum buffer + one big evict
    f32r_transpose=True,   # run PE transposes in float32r (1.5 vs 2 cyc/row)
    dve_inplace=False,     # DVE taps RMW directly into the tap PSUM bank;
                           # ACT evicts PSUM->staging (no DVE merge op)
    dve8=True,             # run DVE taps at 8-row granularity (amortize
                           # per-op overhead across two tap groups)
    tap_bufs=4,            # PSUM buffers for the tap accumulator (1 bank each)
    tout_bufs=2,           # PSUM buffers for the T_out stage
    tin_bufs=1,            # PSUM buffers for the T_in stage (2 banks each)
    xnat_bufs=4,
    stag_bufs=4,
    acc_bufs=4,
    onat_bufs=4,
    xt_bufs=2,
)


def _tap_idx(dy, dx):
    return (dy + 1) * 3 + (dx + 1)


def build_bass(reps=1, **cfg_over):
    cfg = {**DEFAULT_CFG, **cfg_over}
    tdt = F32R if cfg["f32r_transpose"] else F32
    nc = bacc.Bacc(
        "TRN2", target_bir_lowering=False, debug=False, num_devices=N_CORES
    )
    x_d = nc.dram_tensor("x", [ROWS_PER_CORE, C], tdt, kind="ExternalInput").ap()
    # float32r end-to-end for the PE-tap operands: the BIR verifier requires
    # every producer of fp32r-matmul data to round to fp32r.
    diag_d = nc.dram_tensor(
        "diag", [P, 9 * CBLK * P], F32R, kind="ExternalInput"
    ).ap()
    signs_d = nc.dram_tensor("signs", [P, 9 * CBLK], F32, kind="ExternalInput").ap()
    ident_d = nc.dram_tensor("ident", [P, P], tdt, kind="ExternalInput").ap()
    out_d = nc.dram_tensor("out", [ROWS_PER_CORE, C], tdt, kind="ExternalOutput").ap()

    with TileContext(nc) as tc:
        with (
            tc.tile_pool(name="const", bufs=1) as const_pool,
            tc.tile_pool(name="xnat", bufs=cfg["xnat_bufs"]) as xnat_pool,
            tc.tile_pool(name="xT", bufs=cfg["xt_bufs"]) as xT_pool,
            tc.tile_pool(name="acc", bufs=cfg["acc_bufs"]) as acc_pool,
            tc.tile_pool(name="stag", bufs=cfg["stag_bufs"]) as stag_pool,
            tc.tile_pool(name="onat", bufs=cfg["onat_bufs"]) as onat_pool,
            tc.tile_pool(
                name="tinp", bufs=cfg["tin_bufs"], space="PSUM"
            ) as tin_psum,
            tc.tile_pool(
                name="tapp", bufs=cfg["tap_bufs"], space="PSUM"
            ) as tap_psum,
            tc.tile_pool(
                name="toutp",
                bufs=1 if cfg["tout_single"] else cfg["tout_bufs"],
                space="PSUM",
            ) as tout_psum,
        ):
            diag_sb = const_pool.tile([P, 9 * CBLK * P], F32R)
            nc.sync.dma_start(diag_sb[:], diag_d)
            signs_sb = const_pool.tile([P, 9 * CBLK], F32)
            nc.sync.dma_start(signs_sb[:], signs_d)
            ident_sb = const_pool.tile([P, P], tdt)
            nc.sync.dma_start(ident_sb[:], ident_d)

            for _rep in range(reps):
                for img in range(IMG_PER_CORE):
                    for b in range(CBLK):
                        _unit(
                            nc, tc, img, b,
                            x_d, out_d, diag_sb, signs_sb, ident_sb,
                            xnat_pool, xT_pool, acc_pool, stag_pool, onat_pool,
                            tin_psum, tap_psum, tout_psum, cfg,
                        )
    nc.finalize()
    return nc


def _unit(
    nc, tc, img, b,
    x_d, out_d, diag_sb, signs_sb, ident_sb,
    xnat_pool, xT_pool, acc_pool, stag_pool, onat_pool,
    tin_psum, tap_psum, tout_psum, cfg,
):
    nd = cfg["n_dve_taps"]
    nd_list = [nd] * NHG if isinstance(nd, int) else [
        nd[i % len(nd)] for i in range(NHG)
    ]
    tdt = F32R if cfg["f32r_transpose"] else F32
    row0 = img * S

    # ---- transposed, zero-padded x for this (image, channel block) ----
    # Stored as float32r (the PE-tap moving operand must be fp32r-rounded by
    # its producers); DVE taps read it through a plain-f32 bitcast view.
    xT = xT_pool.tile([P, HP * WP], F32R, tag="xT")
    xT3 = xT.rearrange("p (r w) -> p r w", w=WP)  # [128, 114, 114]
    xT3f = xT.bitcast(F32).rearrange("p (r w) -> p r w", w=WP)
    # zero the pad slots: top pad row (h=-1), bottom pad row (h=112),
    # and the two w-pad slots of every row (contiguous pairs at 114k+113).
    # memset can't encode an f32r value in ISA -> write the zero bits as u32.
    xTu3 = xT.bitcast(mybir.dt.uint32).rearrange("p (r w) -> p r w", w=WP)
    nc.vector.memset(xTu3[:, 0, :], 0)
    nc.vector.memset(xTu3[:, HP - 1, :], 0)
    wpads = xT.bitcast(mybir.dt.uint32)[
        :, WP - 1 : WP - 1 + (HP - 1) * WP
    ].rearrange("p (r t) -> p r t", t=WP)[:, :, 0:2]
    nc.vector.memset(wpads, 0)

    # ---- load + transpose-in ----
    for dg in range(DMA_GROUPS):
        xnat = xnat_pool.tile([P, CHUNKS_PER_DMA, P], tdt, tag="xnat")
        src = x_d[
            row0 + dg * ROWS_PER_DMA * W : row0 + (dg + 1) * ROWS_PER_DMA * W,
            b * P : (b + 1) * P,
        ].rearrange("(k p) c -> p k c", p=P)
        nc.sync.dma_start(xnat[:], src)
        for half in range(2):
            g = dg * 2 + half  # row group (8 rows) index, 0..13
            ps_tin = tin_psum.tile([P, ROWG * W], tdt, tag="tin")
            for k in range(CHUNKS_PER_G):
                ck = half * CHUNKS_PER_G + k
                nc.tensor.transpose(
                    ps_tin[:, k * P : (k + 1) * P], xnat[:, ck, :], ident_sb[:]
                )
            # evict into padded rows 8g..8g+7 (padded row index 8g+1..8g+9)
            dst = xT3[:, g * ROWG + 1 : g * ROWG + 1 + ROWG, 1 : 1 + W]
            src_v = ps_tin.rearrange("p (r w) -> p r w", w=W)
            nc.scalar.copy(dst, src_v)

    # ---- taps ----
    stag = None
    for hg in range(NHG):
        n_dve = nd_list[hg]
        dve_taps = TAPS[9 - n_dve :]
        pe_taps = TAPS[: 9 - n_dve]
        h0 = hg * TAP_ROWS
        ps_acc = tap_psum.tile([P, TAP_ROWS * W], F32, tag="tap")
        for i, (dy, dx) in enumerate(pe_taps):
            t = _tap_idx(dy, dx)
            lhsT = diag_sb[:, (t * CBLK + b) * P : (t * CBLK + b + 1) * P]
            rhs = xT3[
                :, h0 + 1 + dy : h0 + 1 + dy + TAP_ROWS, 1 + dx : 1 + dx + W
            ]
            nc.tensor.matmul(
                ps_acc[:],
                lhsT,
                rhs,
                start=(i == 0),
                stop=(i == len(pe_taps) - 1),
            )
        if hg % 2 == 0:
            stag = stag_pool.tile([P, 2 * TAP_ROWS * W], tdt, tag="stag")
        half = hg % 2
        stag_slice = stag[:, half * TAP_ROWS * W : (half + 1) * TAP_ROWS * W]

        def dve_tap_views(i, rows=TAP_ROWS, hh=None):
            dy, dx = dve_taps[i]
            t = _tap_idx(dy, dx)
            sg = signs_sb[:, t * CBLK + b : t * CBLK + b + 1]
            hs = h0 if hh is None else hh
            xs = xT3f[
                :, hs + 1 + dy : hs + 1 + dy + rows, 1 + dx : 1 + dx + W
            ]
            return sg, xs

        if cfg["dve8"] and n_dve > 0:
            # 8-row DVE partial computed once per hg pair
            if half == 0:
                acc8 = acc_pool.tile([P, 2 * TAP_ROWS * W], F32, tag="acc")
                acc8v = acc8.rearrange("p (r w) -> p r w", w=W)
                _unit._acc8 = acc8  # stash on fn (single-threaded build)
                for i in range(n_dve):
                    sg, xs = dve_tap_views(i, rows=2 * TAP_ROWS)
                    if i == 0:
                        if cfg["act_first_mult"]:
                            nc.scalar.mul(acc8v, xs, sg)
                        else:
                            nc.vector.tensor_scalar(
                                acc8v, xs, sg, None, mybir.AluOpType.mult
                            )
                    else:
                        nc.vector.scalar_tensor_tensor(
                            acc8v, xs, sg, acc8v,
                            mybir.AluOpType.mult, mybir.AluOpType.add,
                        )
            acc8 = _unit._acc8
            nc.vector.scalar_tensor_tensor(
                stag_slice,
                ps_acc[:],
                1.0,
                acc8[:, half * TAP_ROWS * W : (half + 1) * TAP_ROWS * W],
                mybir.AluOpType.mult,
                mybir.AluOpType.add,
            )
        elif n_dve == 0:
            # no DVE partial: evict PSUM straight into staging on ACT
            nc.scalar.copy(stag_slice, ps_acc[:])
        elif cfg["dve_inplace"]:
            # DVE taps read-modify-write the PSUM accumulator after the PE
            # group completes; ACT does the final eviction into staging.
            for i in range(n_dve):
                sg, xs = dve_tap_views(i)
                nc.vector.scalar_tensor_tensor(
                    ps_acc[:], xs, sg, ps_acc[:],
                    mybir.AluOpType.mult, mybir.AluOpType.add,
                )
            nc.scalar.copy(stag_slice, ps_acc[:])
        else:
            acc = acc_pool.tile([P, TAP_ROWS * W], F32, tag="acc")
            accv = acc.rearrange("p (r w) -> p r w", w=W)
            for i in range(n_dve):
                sg, xs = dve_tap_views(i)
                if i == 0:
                    if cfg["act_first_mult"]:
                        nc.scalar.mul(accv, xs, sg)
                    else:
                        nc.vector.tensor_scalar(
                            accv, xs, sg, None, mybir.AluOpType.mult
                        )
                else:
                    nc.vector.scalar_tensor_tensor(
                        accv, xs, sg, accv,
                        mybir.AluOpType.mult, mybir.AluOpType.add,
                    )
            # merge PE partial (PSUM) + DVE partial into the staging tile
            nc.vector.scalar_tensor_tensor(
                stag_slice,
                ps_acc[:],
                1.0,
                acc[:],
                mybir.AluOpType.mult,
                mybir.AluOpType.add,
            )
        # ---- transpose-out + evict + store per 8 output rows ----
        if half == 1:
            g = hg // 2
            onat = onat_pool.tile([P, CHUNKS_PER_G, P], tdt, tag="onat")
            if cfg["tout_single"]:
                ps_out = tout_psum.tile([P, CHUNKS_PER_G * P], tdt, tag="tout")
                for k in range(CHUNKS_PER_G):
                    nc.tensor.transpose(
                        ps_out[:, k * P : (k + 1) * P],
                        stag[:, k * P : (k + 1) * P],
                        ident_sb[:],
                    )
                nc.scalar.copy(
                    onat[:],
                    ps_out[:].rearrange("p (k c) -> p k c", c=P),
                )
            else:
                for batch, nchunk in ((0, 4), (1, 3)):
                    ps_out = tout_psum.tile([P, 4 * P], tdt, tag="tout")
                    for k in range(nchunk):
                        ck = batch * 4 + k
                        nc.tensor.transpose(
                            ps_out[:, k * P : (k + 1) * P],
                            stag[:, ck * P : (ck + 1) * P],
                            ident_sb[:],
                        )
                    nc.scalar.copy(
                        onat[:, batch * 4 : batch * 4 + nchunk, :],
                        ps_out[:, : nchunk * P].rearrange("p (k c) -> p k c", c=P),
                    )
            dst = out_d[
                row0 + g * ROWG * W : row0 + (g + 1) * ROWG * W,
                b * P : (b + 1) * P,
            ].rearrange("(k p) c -> p k c", p=P)
            if cfg["out_dma_on_act"]:
                nc.scalar.dma_start(dst, onat[:])
            else:
                nc.sync.dma_start(dst, onat[:])


_NC_CACHE = None


def _get_nc():
    global _NC_CACHE
    if _NC_CACHE is None:
        _NC_CACHE = build_bass()
    return _NC_CACHE


def _host_inputs(w):
    """Per-core constant tensors derived from w (shared by all cores)."""
    signs = np.sign(np.clip(w.astype(np.float32), -1.0, 1.0))[:, :, :, 0]  # (3,3,384)
    signs_flat = signs.reshape(9, C)  # tap-major
    diag = np.zeros((P, 9 * CBLK * P), dtype=np.float32)
    signs_in = np.zeros((P, 9 * CBLK), dtype=np.float32)
    for t in range(9):
        for b in range(CBLK):
            sv = signs_flat[t, b * P : (b + 1) * P]
            col0 = (t * CBLK + b) * P
            diag[np.arange(P), col0 + np.arange(P)] = sv
            signs_in[:, t * CBLK + b] = sv
    ident = np.eye(P, dtype=np.float32)
    return diag, signs_in, ident


def kernel(x, w):
    x = np.asarray(x, dtype=np.float32)
    w = np.asarray(w, dtype=np.float32)
    assert x.shape == (B, H, W, C), x.shape
    nc = _get_nc()
    diag, signs_in, ident = _host_inputs(w)
    in_maps = []
    for core in range(N_CORES):
        xc = x[core * IMG_PER_CORE : (core + 1) * IMG_PER_CORE]
        in_maps.append(
            {
                "x": np.ascontiguousarray(xc.reshape(ROWS_PER_CORE, C)),
                "diag": diag,
                "signs": signs_in,
                "ident": ident,
            }
        )
    res = run_bass_kernel_spmd(nc, in_maps, core_ids=list(range(N_CORES)))
    out = np.empty((B, H, W, C), dtype=np.float32)
    for core in range(N_CORES):
        out[core * IMG_PER_CORE : (core + 1) * IMG_PER_CORE] = res.results[core][
            "out"
        ].reshape(IMG_PER_CORE, H, W, C)
    return out


if __name__ == "__main__":
    rng = np.random.default_rng(0)
    x = rng.standard_normal((B, H, W, C), dtype=np.float32)
    w = rng.standard_normal((3, 3, C, 1), dtype=np.float32)
    out = kernel(x, w)
    print("out", out.shape, out.dtype, float(np.abs(out).mean()))


# revision 35
# speedup vs baseline: 10.2282x; 1.0099x over previous
"""Binary depthwise 3x3 conv (SAME padding) on 8 Trainium2 NeuronCores.

Problem: x (16,112,112,384) f32, w (3,3,384,1) f32.
out[n,h,w,c] = sum_{dy,dx} sign(clip(w))[dy,dx,c] * x[n,h+dy-1,w+dx-1,c]

Strategy (data-parallel, 2 images per core):
  - DMA x in natural NHWC layout (contiguous per partition).
  - PE transpose-mode flips [spatial, c] -> [c, spatial] into a zero-padded
    114-stride row layout so all 9 taps become uniform AP offsets.
  - 7 taps run as float32r diag-matmuls on the PE accumulating in PSUM;
    2 taps run on DVE (tensor_scalar mult + scalar_tensor_tensor), merged
    with the PSUM partial by a final scalar_tensor_tensor.
  - PE transposes back to [spatial, c]; ACT evicts PSUM->SBUF; DMA out.
"""

import os
import sys

sys.path.insert(0, "/opt/trn_rl_repo")

import numpy as np

import concourse.bacc as bacc
import concourse.mybir as mybir
from concourse.tile import TileContext
from concourse.bass_utils import run_bass_kernel_spmd

F32 = mybir.dt.float32
F32R = mybir.dt.float32r

N_CORES = 8
B, H, W, C = 16, 112, 112, 384
IMG_PER_CORE = B // N_CORES          # 2
S = H * W                            # 12544 spatial positions per image
ROWS_PER_CORE = IMG_PER_CORE * S     # 25088
P = 128
CBLK = C // P                        # 3 channel blocks
WP = 114                             # padded row stride (w = -1 .. 112)
HP = 114                             # padded rows (h = -1 .. 112)
ROWG = 8                             # rows per transpose/evict group (8*112 = 7*128)
CHUNKS_PER_G = ROWG * W // P         # 7
NG = H // ROWG                       # 14 row groups per image
DMA_GROUPS = 7                       # input DMAs per (img, cblk): 16 rows each
ROWS_PER_DMA = H // DMA_GROUPS       # 16
CHUNKS_PER_DMA = ROWS_PER_DMA * W // P  # 14
TAP_ROWS = 4                         # output rows per tap matmul (N = 448)
NHG = H // TAP_ROWS                  # 28 tap groups per (img, cblk)

TAPS = [(dy, dx) for dy in (-1, 0, 1) for dx in (-1, 0, 1)]

# tunables (overridable via build_bass kwargs); defaults = best measured
# config from the TimelineSim sweep (333.7 us predicted vs 376 initial)
DEFAULT_CFG = dict(
    n_dve_taps=3,      # taps on DVE (rest on PE); int or per-hg list (cycled)
    act_first_mult=True,   # first DVE tap multiply on ACT instead of DVE
    out_dma_on_act=False,  # issue output DMAs on the ACT HWDGE ring
    tout_single=False,     # single 7-chunk T_out psum buffer + one big evict
    f32r_transpose=True,   # run PE transposes in float32r (1.5 vs 2 cyc/row)
    dve_inplace=False,     # DVE taps RMW directly into the tap PSUM bank;
                           # ACT evicts PSUM->staging (no DVE merge op)
    dve8=True,             # run DVE taps at 8-row granularity (amortize
                           # per-op overhead across two tap groups)
    tap_bufs=3,            # PSUM buffers for the tap accumulator (1 bank each)
    tout_bufs=3,           # PSUM buffers for the T_out stage
    tin_bufs=1,            # PSUM buffers for the T_in stage (2 banks each)
    xnat_bufs=4,
    stag_bufs=4,
    acc_bufs=4,
    onat_bufs=4,
    xt_bufs=2,
    in_dma_on_gpsimd=False,  # issue input DMAs on the SWDGE (gpsimd) path so
                             # the SP HWDGE ring carries only output DMAs
)


def _tap_idx(dy, dx):
    return (dy + 1) * 3 + (dx + 1)


def build_bass(reps=1, **cfg_over):
    cfg = {**DEFAULT_CFG, **cfg_over}
    tdt = F32R if cfg["f32r_transpose"] else F32
    nc = bacc.Bacc(
        "TRN2", target_bir_lowering=False, debug=False, num_devices=N_CORES
    )
    x_d = nc.dram_tensor("x", [ROWS_PER_CORE, C], tdt, kind="ExternalInput").ap()
    # float32r end-to-end for the PE-tap operands: the BIR verifier requires
    # every producer of fp32r-matmul data to round to fp32r.
    diag_d = nc.dram_tensor(
        "diag", [P, 9 * CBLK * P], F32R, kind="ExternalInput"
    ).ap()
    signs_d = nc.dram_tensor("signs", [P, 9 * CBLK], F32, kind="ExternalInput").ap()
    ident_d = nc.dram_tensor("ident", [P, P], tdt, kind="ExternalInput").ap()
    out_d = nc.dram_tensor("out", [ROWS_PER_CORE, C], tdt, kind="ExternalOutput").ap()

    with TileContext(nc) as tc:
        with (
            tc.tile_pool(name="const", bufs=1) as const_pool,
            tc.tile_pool(name="xnat", bufs=cfg["xnat_bufs"]) as xnat_pool,
            tc.tile_pool(name="xT", bufs=cfg["xt_bufs"]) as xT_pool,
            tc.tile_pool(name="acc", bufs=cfg["acc_bufs"]) as acc_pool,
            tc.tile_pool(name="stag", bufs=cfg["stag_bufs"]) as stag_pool,
            tc.tile_pool(name="onat", bufs=cfg["onat_bufs"]) as onat_pool,
            tc.tile_pool(
                name="tinp", bufs=cfg["tin_bufs"], space="PSUM"
            ) as tin_psum,
            tc.tile_pool(
                name="tapp", bufs=cfg["tap_bufs"], space="PSUM"
            ) as tap_psum,
            tc.tile_pool(
                name="toutp",
                bufs=1 if cfg["tout_single"] else cfg["tout_bufs"],
                space="PSUM",
            ) as tout_psum,
        ):
            diag_sb = const_pool.tile([P, 9 * CBLK * P], F32R)
            nc.sync.dma_start(diag_sb[:], diag_d)
            signs_sb = const_pool.tile([P, 9 * CBLK], F32)
            nc.sync.dma_start(signs_sb[:], signs_d)
            ident_sb = const_pool.tile([P, P], tdt)
            nc.sync.dma_start(ident_sb[:], ident_d)

            for _rep in range(reps):
                for img in range(IMG_PER_CORE):
                    for b in range(CBLK):
                        _unit(
                            nc, tc, img, b,
                            x_d, out_d, diag_sb, signs_sb, ident_sb,
                            xnat_pool, xT_pool, acc_pool, stag_pool, onat_pool,
                            tin_psum, tap_psum, tout_psum, cfg,
                        )
    nc.finalize()
    return nc


def _unit(
    nc, tc, img, b,
    x_d, out_d, diag_sb, signs_sb, ident_sb,
    xnat_pool, xT_pool, acc_pool, stag_pool, onat_pool,
    tin_psum, tap_psum, tout_psum, cfg,
):
    nd = cfg["n_dve_taps"]
    nd_list = [nd] * NHG if isinstance(nd, int) else [
        nd[i % len(nd)] for i in range(NHG)
    ]
    tdt = F32R if cfg["f32r_transpose"] else F32
    row0 = img * S

    # ---- transposed, zero-padded x for this (image, channel block) ----
    # Stored as float32r (the PE-tap moving operand must be fp32r-rounded by
    # its producers); DVE taps read it through a plain-f32 bitcast view.
    xT = xT_pool.tile([P, HP * WP], F32R, tag="xT")
    xT3 = xT.rearrange("p (r w) -> p r w", w=WP)  # [128, 114, 114]
    xT3f = xT.bitcast(F32).rearrange("p (r w) -> p r w", w=WP)
    # zero the pad slots: top pad row (h=-1), bottom pad row (h=112),
    # and the two w-pad slots of every row (contiguous pairs at 114k+113).
    # memset can't encode an f32r value in ISA -> write the zero bits as u32.
    xTu3 = xT.bitcast(mybir.dt.uint32).rearrange("p (r w) -> p r w", w=WP)
    nc.vector.memset(xTu3[:, 0, :], 0)
    nc.vector.memset(xTu3[:, HP - 1, :], 0)
    wpads = xT.bitcast(mybir.dt.uint32)[
        :, WP - 1 : WP - 1 + (HP - 1) * WP
    ].rearrange("p (r t) -> p r t", t=WP)[:, :, 0:2]
    nc.vector.memset(wpads, 0)

    # ---- load + transpose-in ----
    for dg in range(DMA_GROUPS):
        xnat = xnat_pool.tile([P, CHUNKS_PER_DMA, P], tdt, tag="xnat")
        src = x_d[
            row0 + dg * ROWS_PER_DMA * W : row0 + (dg + 1) * ROWS_PER_DMA * W,
            b * P : (b + 1) * P,
        ].rearrange("(k p) c -> p k c", p=P)
        if cfg["in_dma_on_gpsimd"]:
            nc.gpsimd.dma_start(xnat[:], src)
        else:
            nc.sync.dma_start(xnat[:], src)
        for half in range(2):
            g = dg * 2 + half  # row group (8 rows) index, 0..13
            ps_tin = tin_psum.tile([P, ROWG * W], tdt, tag="tin")
            for k in range(CHUNKS_PER_G):
                ck = half * CHUNKS_PER_G + k
                nc.tensor.transpose(
                    ps_tin[:, k * P : (k + 1) * P], xnat[:, ck, :], ident_sb[:]
                )
            # evict into padded rows 8g..8g+7 (padded row index 8g+1..8g+9)
            dst = xT3[:, g * ROWG + 1 : g * ROWG + 1 + ROWG, 1 : 1 + W]
            src_v = ps_tin.rearrange("p (r w) -> p r w", w=W)
            nc.scalar.copy(dst, src_v)

    # ---- taps ----
    stag = None
    for hg in range(NHG):
        n_dve = nd_list[hg]
        dve_taps = TAPS[9 - n_dve :]
        pe_taps = TAPS[: 9 - n_dve]
        h0 = hg * TAP_ROWS
        ps_acc = tap_psum.tile([P, TAP_ROWS * W], F32, tag="tap")
        for i, (dy, dx) in enumerate(pe_taps):
            t = _tap_idx(dy, dx)
            lhsT = diag_sb[:, (t * CBLK + b) * P : (t * CBLK + b + 1) * P]
            rhs = xT3[
                :, h0 + 1 + dy : h0 + 1 + dy + TAP_ROWS, 1 + dx : 1 + dx + W
            ]
            nc.tensor.matmul(
                ps_acc[:],
                lhsT,
                rhs,
                start=(i == 0),
                stop=(i == len(pe_taps) - 1),
            )
        if hg % 2 == 0:
            stag = stag_pool.tile([P, 2 * TAP_ROWS * W], tdt, tag="stag")
        half = hg % 2
        stag_slice = stag[:, half * TAP_ROWS * W : (half + 1) * TAP_ROWS * W]

        def dve_tap_views(i, rows=TAP_ROWS, hh=None):
            dy, dx = dve_taps[i]
            t = _tap_idx(dy, dx)
            sg = signs_sb[:, t * CBLK + b : t * CBLK + b + 1]
            hs = h0 if hh is None else hh
            xs = xT3f[
                :, hs + 1 + dy : hs + 1 + dy + rows, 1 + dx : 1 + dx + W
            ]
            return sg, xs

        if cfg["dve8"] and n_dve > 0:
            # 8-row DVE partial computed once per hg pair
            if half == 0:
                acc8 = acc_pool.tile([P, 2 * TAP_ROWS * W], F32, tag="acc")
                acc8v = acc8.rearrange("p (r w) -> p r w", w=W)
                _unit._acc8 = acc8  # stash on fn (single-threaded build)
                for i in range(n_dve):
                    sg, xs = dve_tap_views(i, rows=2 * TAP_ROWS)
                    if i == 0:
                        if cfg["act_first_mult"]:
                            nc.scalar.mul(acc8v, xs, sg)
                        else:
                            nc.vector.tensor_scalar(
                                acc8v, xs, sg, None, mybir.AluOpType.mult
                            )
                    else:
                        nc.vector.scalar_tensor_tensor(
                            acc8v, xs, sg, acc8v,
                            mybir.AluOpType.mult, mybir.AluOpType.add,
                        )
            acc8 = _unit._acc8
            nc.vector.scalar_tensor_tensor(
                stag_slice,
                ps_acc[:],
                1.0,
                acc8[:, half * TAP_ROWS * W : (half + 1) * TAP_ROWS * W],
                mybir.AluOpType.mult,
                mybir.AluOpType.add,
            )
        elif n_dve == 0:
            # no DVE partial: evict PSUM straight into staging on ACT
            nc.scalar.copy(stag_slice, ps_acc[:])
        elif cfg["dve_inplace"]:
            # DVE taps read-modify-write the PSUM accumulator after the PE
            # group completes; ACT does the final eviction into staging.
            for i in range(n_dve):
                sg, xs = dve_tap_views(i)
                nc.vector.scalar_tensor_tensor(
                    ps_acc[:], xs, sg, ps_acc[:],
                    mybir.AluOpType.mult, mybir.AluOpType.add,
                )
            nc.scalar.copy(stag_slice, ps_acc[:])
        else:
            acc = acc_pool.tile([P, TAP_ROWS * W], F32, tag="acc")
            accv = acc.rearrange("p (r w) -> p r w", w=W)
            for i in range(n_dve):
                sg, xs = dve_tap_views(i)
                if i == 0:
                    if cfg["act_first_mult"]:
                        nc.scalar.mul(accv, xs, sg)
                    else:
                        nc.vector.tensor_scalar(
                            accv, xs, sg, None, mybir.AluOpType.mult
                        )
                else:
                    nc.vector.scalar_tensor_tensor(
                        accv, xs, sg, accv,
                        mybir.AluOpType.mult, mybir.AluOpType.add,
                    )
            # merge PE partial (PSUM) + DVE partial into the staging tile
            nc.vector.scalar_tensor_tensor(
                stag_slice,
                ps_acc[:],
                1.0,
                acc[:],
                mybir.AluOpType.mult,
                mybir.AluOpType.add,
            )
        # ---- transpose-out + evict + store per 8 output rows ----
        if half == 1:
            g = hg // 2
            onat = onat_pool.tile([P, CHUNKS_PER_G, P], tdt, tag="onat")
            if cfg["tout_single"]:
                ps_out = tout_psum.tile([P, CHUNKS_PER_G * P], tdt, tag="tout")
                for k in range(CHUNKS_PER_G):
                    nc.tensor.transpose(
                        ps_out[:, k * P : (k + 1) * P],
                        stag[:, k * P : (k + 1) * P],
                        ident_sb[:],
                    )
                nc.scalar.copy(
                    onat[:],
                    ps_out[:].rearrange("p (k c) -> p k c", c=P),
                )
            else:
                for batch, nchunk in ((0, 4), (1, 3)):
                    ps_out = tout_psum.tile([P, 4 * P], tdt, tag="tout")
                    for k in range(nchunk):
                        ck = batch * 4 + k
                        nc.tensor.transpose(
                            ps_out[:, k * P : (k + 1) * P],
                            stag[:, ck * P : (ck + 1) * P],
                            ident_sb[:],
                        )
                    nc.scalar.copy(
                        onat[:, batch * 4 : batch * 4 + nchunk, :],
                        ps_out[:, : nchunk * P].rearrange("p (k c) -> p k c", c=P),
                    )
            dst = out_d[
                row0 + g * ROWG * W : row0 + (g + 1) * ROWG * W,
                b * P : (b + 1) * P,
            ].rearrange("(k p) c -> p k c", p=P)
            if cfg["out_dma_on_act"]:
                nc.scalar.dma_start(dst, onat[:])
            else:
                nc.sync.dma_start(dst, onat[:])


_NC_CACHE = None


def _get_nc():
    global _NC_CACHE
    if _NC_CACHE is None:
        _NC_CACHE = build_bass()
    return _NC_CACHE


def _host_inputs(w):
    """Per-core constant tensors derived from w (shared by all cores)."""
    signs = np.sign(np.clip(w.astype(np.float32), -1.0, 1.0))[:, :, :, 0]  # (3,3,384)
    signs_flat = signs.reshape(9, C)  # tap-major
    diag = np.zeros((P, 9 * CBLK * P), dtype=np.float32)
    signs_in = np.zeros((P, 9 * CBLK), dtype=np.float32)
    for t in range(9):
        for b in range(CBLK):
            sv = signs_flat[t, b * P : (b + 1) * P]
            col0 = (t * CBLK + b) * P
            diag[np.arange(P), col0 + np.arange(P)] = sv
            signs_in[:, t * CBLK + b] = sv
    ident = np.eye(P, dtype=np.float32)
    return diag, signs_in, ident


def kernel(x, w):
    x = np.asarray(x, dtype=np.float32)
    w = np.asarray(w, dtype=np.float32)
    assert x.shape == (B, H, W, C), x.shape
    nc = _get_nc()
    diag, signs_in, ident = _host_inputs(w)
    in_maps = []
    for core in range(N_CORES):
        xc = x[core * IMG_PER_CORE : (core + 1) * IMG_PER_CORE]
        in_maps.append(
            {
                "x": np.ascontiguousarray(xc.reshape(ROWS_PER_CORE, C)),
                "diag": diag,
                "signs": signs_in,
                "ident": ident,
            }
        )
    res = run_bass_kernel_spmd(nc, in_maps, core_ids=list(range(N_CORES)))
    out = np.empty((B, H, W, C), dtype=np.float32)
    for core in range(N_CORES):
        out[core * IMG_PER_CORE : (core + 1) * IMG_PER_CORE] = res.results[core][
            "out"
        ].reshape(IMG_PER_CORE, H, W, C)
    return out


if __name__ == "__main__":
    rng = np.random.default_rng(0)
    x = rng.standard_normal((B, H, W, C), dtype=np.float32)
    w = rng.standard_normal((3, 3, C, 1), dtype=np.float32)
    out = kernel(x, w)
    print("out", out.shape, out.dtype, float(np.abs(out).mean()))
